# revision 3
# baseline (speedup 1.0000x reference)
"""Trainium2 Bass kernel for DigitConvolutionalModel.

Model: x[B,784] -> reshape 28x28 -> 3x3 valid conv (weights conv_w) ->
[B,676] -> Linear(676,100)+relu -> Linear(100,10)+relu -> Linear(10,10).

The conv is linear, so it folds into the first Linear: W1f = C @ w1 where
C[784,676] is the conv unfold matrix. The whole model becomes a 3-layer MLP
784 -> 100 -> 10 -> 10 with relu between layers.

Sharding: pure data parallel, batch split across 8 cores (8192 rows each).

On-chip layout: activations are kept feature-major ([features, batch] on
SBUF partitions) so every matmul consumes weights in their natural [in,out]
layout as the stationary operand:
    h1T[100,n] = sum_k W1f[k,:].T @ xT[k,n]      (K chunks of <=128)
    h2T[10,n]  = w2.T @ relu(h1T+b1)
    y[n,10]    = (relu(h2T+b2) chunk).T @ w3 + b3  (flipped so output is
                                                     batch-major for the store)
x is staged host-side into the feature-major tiled layout the DMA wants:
per 512-batch supertile, [128, 6, 512] (features 0..767, 128 per chunk) plus
a [16, 512] tail (features 768..783), all contiguous per partition.
"""

import numpy as np

import concourse.bacc as bacc
import concourse.tile as tile
from concourse import mybir
from concourse.bass_utils import run_bass_kernel_spmd

N_CORES = 8
B = 65536
BC = B // N_CORES  # 8192 rows per core
TN = 512           # batch columns per supertile
NT = BC // TN      # 16 supertiles per core
NKC = 6            # full 128-feature chunks (0..767)
KT = 16            # tail features (768..783)
NF = 784
H1 = 100
HO = 10
F32 = mybir.dt.float32


def _build_nc():
    nc = bacc.Bacc(None, target_bir_lowering=False)

    xt_main = nc.dram_tensor("xt_main", [NT, 128, NKC, TN], F32, kind="ExternalInput")
    xt_tail = nc.dram_tensor("xt_tail", [NT, KT, TN], F32, kind="ExternalInput")
    w1m = nc.dram_tensor("w1m", [128, NKC, H1], F32, kind="ExternalInput")
    w1t = nc.dram_tensor("w1t", [KT, H1], F32, kind="ExternalInput")
    b1 = nc.dram_tensor("b1", [H1, 1], F32, kind="ExternalInput")
    w2 = nc.dram_tensor("w2", [H1, HO], F32, kind="ExternalInput")
    b2 = nc.dram_tensor("b2", [HO, 1], F32, kind="ExternalInput")
    w3 = nc.dram_tensor("w3", [HO, HO], F32, kind="ExternalInput")
    b3r = nc.dram_tensor("b3r", [128, HO], F32, kind="ExternalInput")
    y = nc.dram_tensor("y", [BC, HO], F32, kind="ExternalOutput")

    relu = mybir.ActivationFunctionType.Relu

    with tile.TileContext(nc) as tc:
        with (
            tc.tile_pool(name="const", bufs=1) as cpool,
            tc.tile_pool(name="io", bufs=3) as iopool,
            tc.tile_pool(name="act", bufs=3) as apool,
            tc.tile_pool(name="ps_a", bufs=2, space="PSUM") as ps_a,
            tc.tile_pool(name="ps_o", bufs=4, space="PSUM") as ps_o,
        ):
            w1m_s = cpool.tile([128, NKC, H1], F32, tag="w1m")
            nc.sync.dma_start(w1m_s[:], w1m[:])
            w1t_s = cpool.tile([KT, H1], F32, tag="w1t")
            nc.sync.dma_start(w1t_s[:], w1t[:])
            b1_s = cpool.tile([H1, 1], F32, tag="b1")
            nc.sync.dma_start(b1_s[:], b1[:])
            w2_s = cpool.tile([H1, HO], F32, tag="w2")
            nc.sync.dma_start(w2_s[:], w2[:])
            b2_s = cpool.tile([HO, 1], F32, tag="b2")
            nc.sync.dma_start(b2_s[:], b2[:])
            w3_s = cpool.tile([HO, HO], F32, tag="w3")
            nc.sync.dma_start(w3_s[:], w3[:])
            b3r_s = cpool.tile([128, HO], F32, tag="b3r")
            nc.sync.dma_start(b3r_s[:], b3r[:])

            for t in range(NT):
                xm = iopool.tile([128, NKC, TN], F32, tag="xm")
                nc.sync.dma_start(xm[:], xt_main[t])
                xtl = iopool.tile([KT, TN], F32, tag="xtl")
                nc.sync.dma_start(xtl[:], xt_tail[t])

                p1 = ps_a.tile([H1, TN], F32, tag="p1")
                for k in range(NKC):
                    nc.tensor.matmul(
                        p1[:], w1m_s[:, k, :], xm[:, k, :],
                        start=(k == 0), stop=False,
                    )
                nc.tensor.matmul(p1[:], w1t_s[:], xtl[:], start=False, stop=True)

                h1 = apool.tile([H1, TN], F32, tag="h1")
                nc.scalar.activation(h1[:], p1[:], relu, bias=b1_s[:, 0:1])

                p2 = ps_a.tile([HO, TN], F32, tag="p2")
                nc.tensor.matmul(p2[:], w2_s[:], h1[:], start=True, stop=True)
                h2 = apool.tile([HO, TN], F32, tag="h2")
                nc.scalar.activation(h2[:], p2[:], relu, bias=b2_s[:, 0:1])

                ot = apool.tile([128, TN // 128, HO], F32, tag="ot")
                for c in range(TN // 128):
                    po = ps_o.tile([128, HO], F32, tag="po")
                    nc.tensor.matmul(
                        po[:], h2[:, c * 128:(c + 1) * 128], w3_s[:],
                        start=True, stop=True,
                    )
                    nc.vector.tensor_add(ot[:, c, :], po[:], b3r_s[:])

                dst = y[t * TN:(t + 1) * TN, :].rearrange("(c p) e -> p c e", p=128)
                nc.sync.dma_start(dst, ot[:])

    nc.compile()
    return nc


def _fold_conv_into_w1(conv_w: np.ndarray, w1: np.ndarray) -> np.ndarray:
    """W1f[784,100] such that x @ W1f == conv(x).reshape(B,676) @ w1."""
    c = np.zeros((NF, 26 * 26), dtype=np.float64)
    for di in range(3):
        for dj in range(3):
            ii, jj = np.meshgrid(np.arange(26), np.arange(26), indexing="ij")
            src = (ii + di) * 28 + (jj + dj)
            dst = ii * 26 + jj
            c[src.ravel(), dst.ravel()] += np.float64(conv_w[di, dj])
    return (c @ w1.astype(np.float64)).astype(np.float32)


def _prep_in_maps(x, conv_w, w1, b1, w2, b2, w3, b3):
    x = np.ascontiguousarray(np.asarray(x, dtype=np.float32))
    conv_w = np.asarray(conv_w, dtype=np.float32)
    w1 = np.asarray(w1, dtype=np.float32)
    b1 = np.asarray(b1, dtype=np.float32)
    w2 = np.asarray(w2, dtype=np.float32)
    b2 = np.asarray(b2, dtype=np.float32)
    w3 = np.asarray(w3, dtype=np.float32)
    b3 = np.asarray(b3, dtype=np.float32)

    w1f = _fold_conv_into_w1(conv_w, w1)  # [784, 100]
    # main chunks: feature f = k*128 + p -> [128, NKC, H1]
    w1m = np.ascontiguousarray(
        w1f[: 128 * NKC].reshape(NKC, 128, H1).transpose(1, 0, 2)
    )
    w1t = np.ascontiguousarray(w1f[128 * NKC:])  # [16, 100]
    b1c = np.ascontiguousarray(b1.reshape(H1, 1))
    b2c = np.ascontiguousarray(b2.reshape(HO, 1))
    b3r = np.ascontiguousarray(np.broadcast_to(b3.reshape(1, HO), (128, HO)))

    shared = {
        "w1m": w1m, "w1t": w1t, "b1": b1c,
        "w2": np.ascontiguousarray(w2), "b2": b2c,
        "w3": np.ascontiguousarray(w3), "b3r": b3r,
    }

    in_maps = []
    for core in range(N_CORES):
        xc = x[core * BC:(core + 1) * BC]  # [8192, 784]
        # [NT, TN, NF] -> feature-major per supertile
        xct = xc.reshape(NT, TN, NF).transpose(0, 2, 1)  # [NT, 784, TN]
        xt_main = np.ascontiguousarray(
            xct[:, : 128 * NKC, :].reshape(NT, NKC, 128, TN).transpose(0, 2, 1, 3)
        )  # [NT, 128, NKC, TN]
        xt_tail = np.ascontiguousarray(xct[:, 128 * NKC:, :])  # [NT, 16, TN]
        in_maps.append({"xt_main": xt_main, "xt_tail": xt_tail, **shared})
    return in_maps


_NC = None


def _get_nc():
    global _NC
    if _NC is None:
        _NC = _build_nc()
    return _NC


def kernel(x, conv_w, w1, b1, w2, b2, w3, b3):
    in_maps = _prep_in_maps(x, conv_w, w1, b1, w2, b2, w3, b3)
    nc = _get_nc()
    res = run_bass_kernel_spmd(nc, in_maps, core_ids=list(range(N_CORES)))
    out = np.concatenate(
        [res.results[i]["y"] for i in range(N_CORES)], axis=0
    )
    return out.astype(np.float32, copy=False)


if __name__ == "__main__":
    rng = np.random.default_rng(0)
    inputs = {
        "x": rng.standard_normal((B, NF), dtype=np.float32),
        "conv_w": np.ones((3, 3), dtype=np.float32),
        "w1": (rng.standard_normal((676, H1)) * 0.04).astype(np.float32),
        "b1": np.zeros(H1, dtype=np.float32),
        "w2": (rng.standard_normal((H1, HO)) * 0.1).astype(np.float32),
        "b2": np.zeros(HO, dtype=np.float32),
        "w3": (rng.standard_normal((HO, HO)) * 0.3).astype(np.float32),
        "b3": np.zeros(HO, dtype=np.float32),
    }
    out = kernel(**inputs)
    print(out.shape, out.dtype)


# revision 4
# speedup vs baseline: 1.6362x; 1.6362x over previous
"""Trainium2 Bass kernel for DigitConvolutionalModel.

Model: x[B,784] -> reshape 28x28 -> 3x3 valid conv (weights conv_w) ->
[B,676] -> Linear(676,100)+relu -> Linear(100,10)+relu -> Linear(10,10).

The conv is linear, so it folds into the first Linear: W1f = C @ w1 where
C[784,676] is the conv unfold matrix. The whole model becomes a 3-layer MLP
784 -> 100 -> 10 -> 10 with relu between layers.

Sharding: pure data parallel, batch split across 8 cores (8192 rows each).

Precision: matmuls in bf16 (PE streams fp32 at 1/4 rate, bf16 at full
rate), accumulation in fp32 PSUM, biases + output in fp32. x is cast to
bf16 host-side — bit-identical to casting on device, but halves the HBM
traffic, which is what the ridge regime wants (DMA ~36us/core vs PE
~30us/core).

On-chip layout: activations are kept feature-major ([features, batch] on
SBUF partitions) so every matmul consumes weights in their natural [in,out]
layout as the stationary operand:
    h1T[100,n] = sum_k W1f[k,:].T @ xT[k,n]      (K chunks of <=128)
    h2T[10,n]  = w2.T @ relu(h1T+b1)
    y[n,10]    = (relu(h2T+b2) chunk).T @ w3 + b3  (flipped so output is
                                                     batch-major for the store)
x is staged host-side into the feature-major tiled layout the DMA wants:
per 512-batch supertile, [128, 6, 512] (features 0..767, 128 per chunk) plus
a [16, 512] tail (features 768..783), all contiguous per partition.
"""

import numpy as np
import ml_dtypes

import concourse.bacc as bacc
import concourse.tile as tile
from concourse import mybir
from concourse.bass_utils import run_bass_kernel_spmd

N_CORES = 8
B = 65536
BC = B // N_CORES  # 8192 rows per core
TN = 512           # batch columns per supertile
NT = BC // TN      # 16 supertiles per core
NKC = 6            # full 128-feature chunks (0..767)
KT = 16            # tail features (768..783)
NF = 784
H1 = 100
HO = 10
F32 = mybir.dt.float32
BF16 = mybir.dt.bfloat16
NP_BF16 = ml_dtypes.bfloat16


def _build_nc():
    nc = bacc.Bacc(None, target_bir_lowering=False)

    xt_main = nc.dram_tensor("xt_main", [NT, 128, NKC, TN], BF16, kind="ExternalInput")
    xt_tail = nc.dram_tensor("xt_tail", [NT, KT, TN], BF16, kind="ExternalInput")
    w1m = nc.dram_tensor("w1m", [128, NKC, H1], BF16, kind="ExternalInput")
    w1t = nc.dram_tensor("w1t", [KT, H1], BF16, kind="ExternalInput")
    b1 = nc.dram_tensor("b1", [H1, 1], F32, kind="ExternalInput")
    w2 = nc.dram_tensor("w2", [H1, HO], BF16, kind="ExternalInput")
    b2 = nc.dram_tensor("b2", [HO, 1], F32, kind="ExternalInput")
    w3 = nc.dram_tensor("w3", [HO, HO], BF16, kind="ExternalInput")
    b3r = nc.dram_tensor("b3r", [128, HO], F32, kind="ExternalInput")
    y = nc.dram_tensor("y", [BC, HO], F32, kind="ExternalOutput")

    relu = mybir.ActivationFunctionType.Relu

    with tile.TileContext(nc) as tc:
        with (
            tc.tile_pool(name="const", bufs=1) as cpool,
            tc.tile_pool(name="io", bufs=3) as iopool,
            tc.tile_pool(name="act", bufs=3) as apool,
            tc.tile_pool(name="ps_a", bufs=2, space="PSUM") as ps_a,
            tc.tile_pool(name="ps_o", bufs=4, space="PSUM") as ps_o,
        ):
            w1m_s = cpool.tile([128, NKC, H1], BF16, tag="w1m")
            nc.sync.dma_start(w1m_s[:], w1m[:])
            w1t_s = cpool.tile([KT, H1], BF16, tag="w1t")
            nc.sync.dma_start(w1t_s[:], w1t[:])
            b1_s = cpool.tile([H1, 1], F32, tag="b1")
            nc.sync.dma_start(b1_s[:], b1[:])
            w2_s = cpool.tile([H1, HO], BF16, tag="w2")
            nc.sync.dma_start(w2_s[:], w2[:])
            b2_s = cpool.tile([HO, 1], F32, tag="b2")
            nc.sync.dma_start(b2_s[:], b2[:])
            w3_s = cpool.tile([HO, HO], BF16, tag="w3")
            nc.sync.dma_start(w3_s[:], w3[:])
            b3r_s = cpool.tile([128, HO], F32, tag="b3r")
            nc.sync.dma_start(b3r_s[:], b3r[:])

            for t in range(NT):
                xm = iopool.tile([128, NKC, TN], BF16, tag="xm")
                nc.sync.dma_start(xm[:], xt_main[t])
                xtl = iopool.tile([KT, TN], BF16, tag="xtl")
                nc.sync.dma_start(xtl[:], xt_tail[t])

                p1 = ps_a.tile([H1, TN], F32, tag="p1")
                for k in range(NKC):
                    nc.tensor.matmul(
                        p1[:], w1m_s[:, k, :], xm[:, k, :],
                        start=(k == 0), stop=False,
                    )
                nc.tensor.matmul(p1[:], w1t_s[:], xtl[:], start=False, stop=True)

                h1 = apool.tile([H1, TN], BF16, tag="h1")
                nc.scalar.activation(h1[:], p1[:], relu, bias=b1_s[:, 0:1])

                p2 = ps_a.tile([HO, TN], F32, tag="p2")
                nc.tensor.matmul(p2[:], w2_s[:], h1[:], start=True, stop=True)
                h2 = apool.tile([HO, TN], BF16, tag="h2")
                nc.scalar.activation(h2[:], p2[:], relu, bias=b2_s[:, 0:1])

                ot = apool.tile([128, TN // 128, HO], F32, tag="ot")
                for c in range(TN // 128):
                    po = ps_o.tile([128, HO], F32, tag="po")
                    nc.tensor.matmul(
                        po[:], h2[:, c * 128:(c + 1) * 128], w3_s[:],
                        start=True, stop=True,
                    )
                    nc.vector.tensor_add(ot[:, c, :], po[:], b3r_s[:])

                dst = y[t * TN:(t + 1) * TN, :].rearrange("(c p) e -> p c e", p=128)
                nc.sync.dma_start(dst, ot[:])

    nc.compile()
    return nc


def _fold_conv_into_w1(conv_w: np.ndarray, w1: np.ndarray) -> np.ndarray:
    """W1f[784,100] such that x @ W1f == conv(x).reshape(B,676) @ w1."""
    c = np.zeros((NF, 26 * 26), dtype=np.float64)
    for di in range(3):
        for dj in range(3):
            ii, jj = np.meshgrid(np.arange(26), np.arange(26), indexing="ij")
            src = (ii + di) * 28 + (jj + dj)
            dst = ii * 26 + jj
            c[src.ravel(), dst.ravel()] += np.float64(conv_w[di, dj])
    return (c @ w1.astype(np.float64)).astype(np.float32)


def _prep_in_maps(x, conv_w, w1, b1, w2, b2, w3, b3):
    x = np.asarray(x, dtype=np.float32)
    conv_w = np.asarray(conv_w, dtype=np.float32)
    w1 = np.asarray(w1, dtype=np.float32)
    b1 = np.asarray(b1, dtype=np.float32)
    w2 = np.asarray(w2, dtype=np.float32)
    b2 = np.asarray(b2, dtype=np.float32)
    w3 = np.asarray(w3, dtype=np.float32)
    b3 = np.asarray(b3, dtype=np.float32)

    w1f = _fold_conv_into_w1(conv_w, w1)  # [784, 100]
    # main chunks: feature f = k*128 + p -> [128, NKC, H1]
    w1m = np.ascontiguousarray(
        w1f[: 128 * NKC].reshape(NKC, 128, H1).transpose(1, 0, 2)
    ).astype(NP_BF16)
    w1t = np.ascontiguousarray(w1f[128 * NKC:]).astype(NP_BF16)  # [16, 100]
    b1c = np.ascontiguousarray(b1.reshape(H1, 1))
    b2c = np.ascontiguousarray(b2.reshape(HO, 1))
    b3r = np.ascontiguousarray(np.broadcast_to(b3.reshape(1, HO), (128, HO)))

    shared = {
        "w1m": w1m, "w1t": w1t, "b1": b1c,
        "w2": np.ascontiguousarray(w2).astype(NP_BF16), "b2": b2c,
        "w3": np.ascontiguousarray(w3).astype(NP_BF16), "b3r": b3r,
    }

    xb = x.astype(NP_BF16)  # cast once, full batch
    in_maps = []
    for core in range(N_CORES):
        xc = xb[core * BC:(core + 1) * BC]  # [8192, 784] bf16
        # [NT, TN, NF] -> feature-major per supertile
        xct = xc.reshape(NT, TN, NF).transpose(0, 2, 1)  # [NT, 784, TN]
        xt_main = np.ascontiguousarray(
            xct[:, : 128 * NKC, :].reshape(NT, NKC, 128, TN).transpose(0, 2, 1, 3)
        )  # [NT, 128, NKC, TN]
        xt_tail = np.ascontiguousarray(xct[:, 128 * NKC:, :])  # [NT, 16, TN]
        in_maps.append({"xt_main": xt_main, "xt_tail": xt_tail, **shared})
    return in_maps


_NC = None


def _get_nc():
    global _NC
    if _NC is None:
        _NC = _build_nc()
    return _NC


def kernel(x, conv_w, w1, b1, w2, b2, w3, b3):
    in_maps = _prep_in_maps(x, conv_w, w1, b1, w2, b2, w3, b3)
    nc = _get_nc()
    res = run_bass_kernel_spmd(nc, in_maps, core_ids=list(range(N_CORES)))
    out = np.concatenate(
        [res.results[i]["y"] for i in range(N_CORES)], axis=0
    )
    return out.astype(np.float32, copy=False)


if __name__ == "__main__":
    rng = np.random.default_rng(0)
    inputs = {
        "x": rng.standard_normal((B, NF), dtype=np.float32),
        "conv_w": np.ones((3, 3), dtype=np.float32),
        "w1": (rng.standard_normal((676, H1)) * 0.04).astype(np.float32),
        "b1": np.zeros(H1, dtype=np.float32),
        "w2": (rng.standard_normal((H1, HO)) * 0.1).astype(np.float32),
        "b2": np.zeros(HO, dtype=np.float32),
        "w3": (rng.standard_normal((HO, HO)) * 0.3).astype(np.float32),
        "b3": np.zeros(HO, dtype=np.float32),
    }
    out = kernel(**inputs)
    print(out.shape, out.dtype)


# revision 6
# speedup vs baseline: 2.1199x; 1.2956x over previous
"""Trainium2 Bass kernel for DigitConvolutionalModel.

Model: x[B,784] -> reshape 28x28 -> 3x3 valid conv (weights conv_w) ->
[B,676] -> Linear(676,100)+relu -> Linear(100,10)+relu -> Linear(10,10).

The conv is linear, so it folds into the first Linear: W1f = C @ w1 where
C[784,676] is the conv unfold matrix. The whole model becomes a 3-layer MLP
784 -> 100 -> 10 -> 10 with relu between layers.

Sharding: pure data parallel, batch split across 8 cores (8192 rows each).

Precision: matmuls in bf16 (PE streams fp32 at 1/4 rate, bf16 at full
rate), accumulation in fp32 PSUM, biases + output in fp32. x is cast to
bf16 host-side — bit-identical to casting on device, but halves the HBM
traffic, which is what the ridge regime wants (DMA ~36us/core vs PE
~31us/core).

On-chip layout: activations stay feature-major ([features, batch] on SBUF
partitions) end to end, so every matmul uses the weights in natural [in,out]
layout as the stationary operand and the batch streams as the moving free
dim:
    h1T[100,n] = sum_k W1f[k,:].T @ xT[k,n]      (K chunks of <=128)
    h2T[10,n]  = w2.T @ relu(h1T+b1)
    yT[10,n]   = w3.T @ relu(h2T+b2) + b3
The [10, B] output is stored feature-major (2KB-contiguous DMA runs; a
batch-major store would emit 40-byte descriptors) and transposed on host.

x is staged host-side into the feature-major tiled layout the DMA wants:
per 512-batch supertile [128, 6, 512] (features 0..767, 6KB contiguous per
partition); the [16, 512] feature tails (768..783) of all supertiles are
preloaded in one DMA.
"""

import numpy as np
import ml_dtypes

import concourse.bacc as bacc
import concourse.tile as tile
from concourse import mybir
from concourse.bass_utils import run_bass_kernel_spmd

N_CORES = 8
B = 65536
BC = B // N_CORES  # 8192 rows per core
TN = 512           # batch columns per supertile
NT = BC // TN      # 16 supertiles per core
NKC = 6            # full 128-feature chunks (0..767)
KT = 16            # tail features (768..783)
NF = 784
H1 = 100
HO = 10
F32 = mybir.dt.float32
BF16 = mybir.dt.bfloat16
NP_BF16 = ml_dtypes.bfloat16


def _build_nc():
    nc = bacc.Bacc(None, target_bir_lowering=False)

    xt_main = nc.dram_tensor("xt_main", [NT, 128, NKC, TN], BF16, kind="ExternalInput")
    xt_tail = nc.dram_tensor("xt_tail", [KT, NT, TN], BF16, kind="ExternalInput")
    w1m = nc.dram_tensor("w1m", [128, NKC, H1], BF16, kind="ExternalInput")
    w1t = nc.dram_tensor("w1t", [KT, H1], BF16, kind="ExternalInput")
    b1 = nc.dram_tensor("b1", [H1, 1], F32, kind="ExternalInput")
    w2 = nc.dram_tensor("w2", [H1, HO], BF16, kind="ExternalInput")
    b2 = nc.dram_tensor("b2", [HO, 1], F32, kind="ExternalInput")
    w3 = nc.dram_tensor("w3", [HO, HO], BF16, kind="ExternalInput")
    b3w = nc.dram_tensor("b3w", [HO, TN], F32, kind="ExternalInput")
    yt = nc.dram_tensor("yt", [HO, BC], F32, kind="ExternalOutput")

    relu = mybir.ActivationFunctionType.Relu

    with tile.TileContext(nc) as tc:
        with (
            tc.tile_pool(name="const", bufs=1) as cpool,
            tc.tile_pool(name="io", bufs=4) as iopool,
            tc.tile_pool(name="act", bufs=3) as apool,
            tc.tile_pool(name="ps1", bufs=3, space="PSUM") as ps1,
            tc.tile_pool(name="ps2", bufs=2, space="PSUM") as ps2,
            tc.tile_pool(name="ps3", bufs=2, space="PSUM") as ps3,
        ):
            w1m_s = cpool.tile([128, NKC, H1], BF16, tag="w1m")
            nc.sync.dma_start(w1m_s[:], w1m[:])
            w1t_s = cpool.tile([KT, H1], BF16, tag="w1t")
            nc.sync.dma_start(w1t_s[:], w1t[:])
            xtl_s = cpool.tile([KT, NT, TN], BF16, tag="xtl")
            nc.sync.dma_start(xtl_s[:], xt_tail[:])
            b1_s = cpool.tile([H1, 1], F32, tag="b1")
            nc.sync.dma_start(b1_s[:], b1[:])
            w2_s = cpool.tile([H1, HO], BF16, tag="w2")
            nc.sync.dma_start(w2_s[:], w2[:])
            b2_s = cpool.tile([HO, 1], F32, tag="b2")
            nc.sync.dma_start(b2_s[:], b2[:])
            w3_s = cpool.tile([HO, HO], BF16, tag="w3")
            nc.sync.dma_start(w3_s[:], w3[:])
            b3w_s = cpool.tile([HO, TN], F32, tag="b3w")
            nc.sync.dma_start(b3w_s[:], b3w[:])

            # Software pipeline: at step t, emit L1(t), L2(t-1), L3(t-2)
            # so the PE never waits on a just-issued ACT relu — each PE
            # instruction's input is a full L1-block (~3us) old.
            h1s: dict[int, object] = {}
            h2s: dict[int, object] = {}
            for t in range(NT + 2):
                if t < NT:
                    xm = iopool.tile([128, NKC, TN], BF16, tag="xm")
                    nc.sync.dma_start(xm[:], xt_main[t])

                    p1 = ps1.tile([H1, TN], F32, tag="p1")
                    for k in range(NKC):
                        nc.tensor.matmul(
                            p1[:], w1m_s[:, k, :], xm[:, k, :],
                            start=(k == 0), stop=False,
                        )
                    nc.tensor.matmul(
                        p1[:], w1t_s[:], xtl_s[:, t, :], start=False, stop=True
                    )
                    h1 = apool.tile([H1, TN], BF16, tag="h1")
                    nc.scalar.activation(h1[:], p1[:], relu, bias=b1_s[:, 0:1])
                    h1s[t] = h1

                if 1 <= t <= NT:
                    s = t - 1
                    p2 = ps2.tile([HO, TN], F32, tag="p2")
                    nc.tensor.matmul(
                        p2[:], w2_s[:], h1s.pop(s)[:], start=True, stop=True
                    )
                    h2 = apool.tile([HO, TN], BF16, tag="h2")
                    nc.scalar.activation(h2[:], p2[:], relu, bias=b2_s[:, 0:1])
                    h2s[s] = h2

                if t >= 2:
                    s = t - 2
                    p3 = ps3.tile([HO, TN], F32, tag="p3")
                    nc.tensor.matmul(
                        p3[:], w3_s[:], h2s.pop(s)[:], start=True, stop=True
                    )
                    ot = apool.tile([HO, TN], F32, tag="ot")
                    nc.vector.tensor_add(ot[:], p3[:], b3w_s[:])
                    nc.sync.dma_start(yt[:, s * TN:(s + 1) * TN], ot[:])

    nc.compile()
    return nc


def _fold_conv_into_w1(conv_w: np.ndarray, w1: np.ndarray) -> np.ndarray:
    """W1f[784,100] such that x @ W1f == conv(x).reshape(B,676) @ w1."""
    c = np.zeros((NF, 26 * 26), dtype=np.float64)
    for di in range(3):
        for dj in range(3):
            ii, jj = np.meshgrid(np.arange(26), np.arange(26), indexing="ij")
            src = (ii + di) * 28 + (jj + dj)
            dst = ii * 26 + jj
            c[src.ravel(), dst.ravel()] += np.float64(conv_w[di, dj])
    return (c @ w1.astype(np.float64)).astype(np.float32)


def _prep_in_maps(x, conv_w, w1, b1, w2, b2, w3, b3):
    x = np.asarray(x, dtype=np.float32)
    conv_w = np.asarray(conv_w, dtype=np.float32)
    w1 = np.asarray(w1, dtype=np.float32)
    b1 = np.asarray(b1, dtype=np.float32)
    w2 = np.asarray(w2, dtype=np.float32)
    b2 = np.asarray(b2, dtype=np.float32)
    w3 = np.asarray(w3, dtype=np.float32)
    b3 = np.asarray(b3, dtype=np.float32)

    w1f = _fold_conv_into_w1(conv_w, w1)  # [784, 100]
    # main chunks: feature f = k*128 + p -> [128, NKC, H1]
    w1m = np.ascontiguousarray(
        w1f[: 128 * NKC].reshape(NKC, 128, H1).transpose(1, 0, 2)
    ).astype(NP_BF16)
    w1t = np.ascontiguousarray(w1f[128 * NKC:]).astype(NP_BF16)  # [16, 100]
    b1c = np.ascontiguousarray(b1.reshape(H1, 1))
    b2c = np.ascontiguousarray(b2.reshape(HO, 1))
    b3w = np.ascontiguousarray(np.broadcast_to(b3.reshape(HO, 1), (HO, TN)))

    shared = {
        "w1m": w1m, "w1t": w1t, "b1": b1c,
        "w2": np.ascontiguousarray(w2).astype(NP_BF16), "b2": b2c,
        "w3": np.ascontiguousarray(w3).astype(NP_BF16), "b3w": b3w,
    }

    xb = x.astype(NP_BF16)  # cast once, full batch
    in_maps = []
    for core in range(N_CORES):
        xc = xb[core * BC:(core + 1) * BC]  # [8192, 784] bf16
        # [NT, TN, NF] -> feature-major per supertile
        xct = xc.reshape(NT, TN, NF).transpose(0, 2, 1)  # [NT, 784, TN]
        xt_main = np.ascontiguousarray(
            xct[:, : 128 * NKC, :].reshape(NT, NKC, 128, TN).transpose(0, 2, 1, 3)
        )  # [NT, 128, NKC, TN]
        # tails of all supertiles together: [KT, NT, TN]
        xt_tail = np.ascontiguousarray(xct[:, 128 * NKC:, :].transpose(1, 0, 2))
        in_maps.append({"xt_main": xt_main, "xt_tail": xt_tail, **shared})
    return in_maps


_NC = None


def _get_nc():
    global _NC
    if _NC is None:
        _NC = _build_nc()
    return _NC


def kernel(x, conv_w, w1, b1, w2, b2, w3, b3):
    in_maps = _prep_in_maps(x, conv_w, w1, b1, w2, b2, w3, b3)
    nc = _get_nc()
    res = run_bass_kernel_spmd(nc, in_maps, core_ids=list(range(N_CORES)))
    out = np.empty((B, HO), dtype=np.float32)
    for i in range(N_CORES):
        out[i * BC:(i + 1) * BC] = res.results[i]["yt"].T
    return out


if __name__ == "__main__":
    rng = np.random.default_rng(0)
    inputs = {
        "x": rng.standard_normal((B, NF), dtype=np.float32),
        "conv_w": np.ones((3, 3), dtype=np.float32),
        "w1": (rng.standard_normal((676, H1)) * 0.04).astype(np.float32),
        "b1": np.zeros(H1, dtype=np.float32),
        "w2": (rng.standard_normal((H1, HO)) * 0.1).astype(np.float32),
        "b2": np.zeros(HO, dtype=np.float32),
        "w3": (rng.standard_normal((HO, HO)) * 0.3).astype(np.float32),
        "b3": np.zeros(HO, dtype=np.float32),
    }
    out = kernel(**inputs)
    print(out.shape, out.dtype)


# revision 9
# speedup vs baseline: 2.1274x; 1.0036x over previous
"""Trainium2 Bass kernel for DigitConvolutionalModel.

Model: x[B,784] -> reshape 28x28 -> 3x3 valid conv (weights conv_w) ->
[B,676] -> Linear(676,100)+relu -> Linear(100,10)+relu -> Linear(10,10).

The conv is linear, so it folds into the first Linear: W1f = C @ w1 where
C[784,676] is the conv unfold matrix. The whole model becomes a 3-layer MLP
784 -> 100 -> 10 -> 10 with relu between layers.

Sharding: pure data parallel, batch split across 8 cores (8192 rows each).

Precision: matmuls in bf16 (PE streams fp32 at 1/4 rate, bf16 at full
rate), accumulation in fp32 PSUM, biases + output in fp32. x is cast to
bf16 host-side — bit-identical to casting on device, but halves the HBM
traffic, which is what the ridge regime wants (DMA ~36us/core vs PE
~31us/core).

On-chip layout: activations stay feature-major ([features, batch] on SBUF
partitions) end to end, so every matmul uses the weights in natural [in,out]
layout as the stationary operand and the batch streams as the moving free
dim:
    h1T[100,n] = sum_k W1f[k,:].T @ xT[k,n]      (K chunks of <=128)
    h2T[10,n]  = w2.T @ relu(h1T+b1)
    yT[10,n]   = w3.T @ relu(h2T+b2) + b3
The [10, B] output is stored feature-major (2KB-contiguous DMA runs; a
batch-major store would emit 40-byte descriptors) and transposed on host.

x is staged host-side into the feature-major tiled layout the DMA wants:
per 512-batch supertile [128, 6, 512] (features 0..767, 6KB contiguous per
partition); the [16, 512] feature tails (768..783) of all supertiles are
preloaded in one DMA.
"""

import numpy as np
import ml_dtypes

import concourse.bacc as bacc
import concourse.tile as tile
from concourse import mybir
from concourse.bass_utils import run_bass_kernel_spmd

N_CORES = 8
B = 65536
BC = B // N_CORES  # 8192 rows per core
TN = 512           # batch columns per supertile
NT = BC // TN      # 16 supertiles per core
NKC = 6            # full 128-feature chunks (0..767)
KT = 16            # tail features (768..783)
NF = 784
H1 = 100
HO = 10
F32 = mybir.dt.float32
BF16 = mybir.dt.bfloat16
NP_BF16 = ml_dtypes.bfloat16


def _build_nc():
    nc = bacc.Bacc(None, target_bir_lowering=False)

    xt_main = nc.dram_tensor("xt_main", [NT, 128, NKC, TN], BF16, kind="ExternalInput")
    xt_tail = nc.dram_tensor("xt_tail", [KT, NT, TN], BF16, kind="ExternalInput")
    w1m = nc.dram_tensor("w1m", [128, NKC, H1], BF16, kind="ExternalInput")
    w1t = nc.dram_tensor("w1t", [KT, H1], BF16, kind="ExternalInput")
    b1 = nc.dram_tensor("b1", [H1, 1], F32, kind="ExternalInput")
    w2 = nc.dram_tensor("w2", [H1, HO], BF16, kind="ExternalInput")
    b2 = nc.dram_tensor("b2", [HO, 1], F32, kind="ExternalInput")
    w3 = nc.dram_tensor("w3", [HO, HO], BF16, kind="ExternalInput")
    b3w = nc.dram_tensor("b3w", [HO, TN], F32, kind="ExternalInput")
    yt = nc.dram_tensor("yt", [HO, BC], F32, kind="ExternalOutput")

    relu = mybir.ActivationFunctionType.Relu

    with tile.TileContext(nc) as tc:
        with (
            tc.tile_pool(name="const", bufs=1) as cpool,
            tc.tile_pool(name="io", bufs=4) as iopool,
            tc.tile_pool(name="act", bufs=4) as apool,
            tc.tile_pool(name="ps1", bufs=3, space="PSUM") as ps1,
            tc.tile_pool(name="ps2", bufs=2, space="PSUM") as ps2,
            tc.tile_pool(name="ps3", bufs=2, space="PSUM") as ps3,
        ):
# Weights/consts go on the scalar HWDGE queue-set so they don't
            # delay the batch-data stream on the sync queue-set.
            w1m_s = cpool.tile([128, NKC, H1], BF16, tag="w1m")
            nc.scalar.dma_start(w1m_s[:], w1m[:])
            w1t_s = cpool.tile([KT, H1], BF16, tag="w1t")
            nc.scalar.dma_start(w1t_s[:], w1t[:])
            xtl_s = cpool.tile([KT, NT, TN], BF16, tag="xtl")
            nc.scalar.dma_start(xtl_s[:], xt_tail[:])
            b1_s = cpool.tile([H1, 1], F32, tag="b1")
            nc.scalar.dma_start(b1_s[:], b1[:])
            w2_s = cpool.tile([H1, HO], BF16, tag="w2")
            nc.scalar.dma_start(w2_s[:], w2[:])
            b2_s = cpool.tile([HO, 1], F32, tag="b2")
            nc.scalar.dma_start(b2_s[:], b2[:])
            w3_s = cpool.tile([HO, HO], BF16, tag="w3")
            nc.scalar.dma_start(w3_s[:], w3[:])
            b3w_s = cpool.tile([HO, TN], F32, tag="b3w")
            nc.scalar.dma_start(b3w_s[:], b3w[:])

            # Software pipeline: at step t, emit L1(t), L2(t-2), L3(t-4)
            # so each PE instruction's input was produced >= 2 full
            # L1-blocks earlier — the PE never waits on a fresh ACT relu.
            L2_LAG, L3_LAG = 2, 4
            h1s: dict[int, object] = {}
            h2s: dict[int, object] = {}
            for t in range(NT + L3_LAG):
                if t < NT:
                    xm = iopool.tile([128, NKC, TN], BF16, tag="xm")
                    nc.sync.dma_start(xm[:], xt_main[t])

                    p1 = ps1.tile([H1, TN], F32, tag="p1")
                    for k in range(NKC):
                        nc.tensor.matmul(
                            p1[:], w1m_s[:, k, :], xm[:, k, :],
                            start=(k == 0), stop=False,
                        )
                    nc.tensor.matmul(
                        p1[:], w1t_s[:], xtl_s[:, t, :], start=False, stop=True
                    )
                    h1 = apool.tile([H1, TN], BF16, tag="h1")
                    nc.scalar.activation(h1[:], p1[:], relu, bias=b1_s[:, 0:1])
                    h1s[t] = h1

                if L2_LAG <= t < NT + L2_LAG:
                    s = t - L2_LAG
                    p2 = ps2.tile([HO, TN], F32, tag="p2")
                    nc.tensor.matmul(
                        p2[:], w2_s[:], h1s.pop(s)[:], start=True, stop=True
                    )
                    h2 = apool.tile([HO, TN], BF16, tag="h2")
                    nc.scalar.activation(h2[:], p2[:], relu, bias=b2_s[:, 0:1])
                    h2s[s] = h2

                if t >= L3_LAG:
                    s = t - L3_LAG
                    p3 = ps3.tile([HO, TN], F32, tag="p3")
                    nc.tensor.matmul(
                        p3[:], w3_s[:], h2s.pop(s)[:], start=True, stop=True
                    )
                    ot = apool.tile([HO, TN], F32, tag="ot")
                    nc.vector.tensor_add(ot[:], p3[:], b3w_s[:])
                    nc.sync.dma_start(yt[:, s * TN:(s + 1) * TN], ot[:])

    nc.compile()
    return nc


def _fold_conv_into_w1(conv_w: np.ndarray, w1: np.ndarray) -> np.ndarray:
    """W1f[784,100] such that x @ W1f == conv(x).reshape(B,676) @ w1."""
    c = np.zeros((NF, 26 * 26), dtype=np.float64)
    for di in range(3):
        for dj in range(3):
            ii, jj = np.meshgrid(np.arange(26), np.arange(26), indexing="ij")
            src = (ii + di) * 28 + (jj + dj)
            dst = ii * 26 + jj
            c[src.ravel(), dst.ravel()] += np.float64(conv_w[di, dj])
    return (c @ w1.astype(np.float64)).astype(np.float32)


def _prep_in_maps(x, conv_w, w1, b1, w2, b2, w3, b3):
    x = np.asarray(x, dtype=np.float32)
    conv_w = np.asarray(conv_w, dtype=np.float32)
    w1 = np.asarray(w1, dtype=np.float32)
    b1 = np.asarray(b1, dtype=np.float32)
    w2 = np.asarray(w2, dtype=np.float32)
    b2 = np.asarray(b2, dtype=np.float32)
    w3 = np.asarray(w3, dtype=np.float32)
    b3 = np.asarray(b3, dtype=np.float32)

    w1f = _fold_conv_into_w1(conv_w, w1)  # [784, 100]
    # main chunks: feature f = k*128 + p -> [128, NKC, H1]
    w1m = np.ascontiguousarray(
        w1f[: 128 * NKC].reshape(NKC, 128, H1).transpose(1, 0, 2)
    ).astype(NP_BF16)
    w1t = np.ascontiguousarray(w1f[128 * NKC:]).astype(NP_BF16)  # [16, 100]
    b1c = np.ascontiguousarray(b1.reshape(H1, 1))
    b2c = np.ascontiguousarray(b2.reshape(HO, 1))
    b3w = np.ascontiguousarray(np.broadcast_to(b3.reshape(HO, 1), (HO, TN)))

    shared = {
        "w1m": w1m, "w1t": w1t, "b1": b1c,
        "w2": np.ascontiguousarray(w2).astype(NP_BF16), "b2": b2c,
        "w3": np.ascontiguousarray(w3).astype(NP_BF16), "b3w": b3w,
    }

    xb = x.astype(NP_BF16)  # cast once, full batch
    in_maps = []
    for core in range(N_CORES):
        xc = xb[core * BC:(core + 1) * BC]  # [8192, 784] bf16
        # [NT, TN, NF] -> feature-major per supertile
        xct = xc.reshape(NT, TN, NF).transpose(0, 2, 1)  # [NT, 784, TN]
        xt_main = np.ascontiguousarray(
            xct[:, : 128 * NKC, :].reshape(NT, NKC, 128, TN).transpose(0, 2, 1, 3)
        )  # [NT, 128, NKC, TN]
        # tails of all supertiles together: [KT, NT, TN]
        xt_tail = np.ascontiguousarray(xct[:, 128 * NKC:, :].transpose(1, 0, 2))
        in_maps.append({"xt_main": xt_main, "xt_tail": xt_tail, **shared})
    return in_maps


_NC = None


def _get_nc():
    global _NC
    if _NC is None:
        _NC = _build_nc()
    return _NC


def kernel(x, conv_w, w1, b1, w2, b2, w3, b3):
    in_maps = _prep_in_maps(x, conv_w, w1, b1, w2, b2, w3, b3)
    nc = _get_nc()
    res = run_bass_kernel_spmd(nc, in_maps, core_ids=list(range(N_CORES)))
    out = np.empty((B, HO), dtype=np.float32)
    for i in range(N_CORES):
        out[i * BC:(i + 1) * BC] = res.results[i]["yt"].T
    return out


if __name__ == "__main__":
    rng = np.random.default_rng(0)
    inputs = {
        "x": rng.standard_normal((B, NF), dtype=np.float32),
        "conv_w": np.ones((3, 3), dtype=np.float32),
        "w1": (rng.standard_normal((676, H1)) * 0.04).astype(np.float32),
        "b1": np.zeros(H1, dtype=np.float32),
        "w2": (rng.standard_normal((H1, HO)) * 0.1).astype(np.float32),
        "b2": np.zeros(HO, dtype=np.float32),
        "w3": (rng.standard_normal((HO, HO)) * 0.3).astype(np.float32),
        "b3": np.zeros(HO, dtype=np.float32),
    }
    out = kernel(**inputs)
    print(out.shape, out.dtype)


# revision 12
# speedup vs baseline: 2.3716x; 1.1148x over previous
"""Trainium2 Bass kernel for DigitConvolutionalModel.

Model: x[B,784] -> reshape 28x28 -> 3x3 valid conv (weights conv_w) ->
[B,676] -> Linear(676,100)+relu -> Linear(100,10)+relu -> Linear(10,10).

The conv is linear, so it folds into the first Linear: W1f = C @ w1 where
C[784,676] is the conv unfold matrix. The whole model becomes a 3-layer MLP
784 -> 100 -> 10 -> 10 with relu between layers.

Sharding: pure data parallel, batch split across 8 cores (8192 rows each).

Precision: matmuls in bf16 (PE streams fp32 at 1/4 rate, bf16 at full
rate), accumulation in fp32 PSUM, biases + output in fp32. x is cast to
bf16 host-side — bit-identical to casting on device, but halves the HBM
traffic, which is what the ridge regime wants (DMA ~36us/core vs PE
~31us/core).

On-chip layout: activations stay feature-major ([features, batch] on SBUF
partitions) end to end, so every matmul uses the weights in natural [in,out]
layout as the stationary operand and the batch streams as the moving free
dim:
    h1T[100,n] = sum_k W1f[k,:].T @ xT[k,n]      (K chunks of <=128)
    h2T[10,n]  = w2.T @ relu(h1T+b1)
    yT[10,n]   = w3.T @ relu(h2T+b2) + b3
The [10, B] output is stored feature-major (2KB-contiguous DMA runs; a
batch-major store would emit 40-byte descriptors) and transposed on host.

x is staged host-side into the feature-major tiled layout the DMA wants:
per 512-batch supertile [128, 6, 512] (features 0..767, 6KB contiguous per
partition); the [16, 512] feature tails (768..783) of all supertiles are
preloaded in one DMA.
"""

import numpy as np
import ml_dtypes

import concourse.bacc as bacc
import concourse.tile as tile
from concourse import mybir
from concourse.bass_utils import run_bass_kernel_spmd

N_CORES = 8
B = 65536
BC = B // N_CORES  # 8192 rows per core
TN = 512           # batch columns per supertile
NT = BC // TN      # 16 supertiles per core
NKC = 6            # full 128-feature chunks (0..767)
KT = 16            # tail features (768..783)
NF = 784
H1 = 100
HO = 10
F32 = mybir.dt.float32
BF16 = mybir.dt.bfloat16
NP_BF16 = ml_dtypes.bfloat16


def _build_nc():
    nc = bacc.Bacc(None, target_bir_lowering=False)

    xt_main = nc.dram_tensor("xt_main", [NT, 128, NKC, TN], BF16, kind="ExternalInput")
    xt_tail = nc.dram_tensor("xt_tail", [KT, NT, TN], BF16, kind="ExternalInput")
    w1m = nc.dram_tensor("w1m", [128, NKC, H1], BF16, kind="ExternalInput")
    w1t = nc.dram_tensor("w1t", [KT, H1], BF16, kind="ExternalInput")
    b1 = nc.dram_tensor("b1", [H1, 1], F32, kind="ExternalInput")
    w2 = nc.dram_tensor("w2", [H1, HO], BF16, kind="ExternalInput")
    b2 = nc.dram_tensor("b2", [HO, 1], F32, kind="ExternalInput")
    w3 = nc.dram_tensor("w3", [HO, HO], BF16, kind="ExternalInput")
    b3w = nc.dram_tensor("b3w", [HO, TN], F32, kind="ExternalInput")
    yt = nc.dram_tensor("yt", [HO, BC], F32, kind="ExternalOutput")

    relu = mybir.ActivationFunctionType.Relu

    with tile.TileContext(nc) as tc:
        with (
            tc.tile_pool(name="const", bufs=1) as cpool,
            tc.tile_pool(name="io", bufs=4) as iopool,
            tc.tile_pool(name="act", bufs=4) as apool,
            tc.tile_pool(name="ps1", bufs=4, space="PSUM") as ps1,
            tc.tile_pool(name="ps2", bufs=2, space="PSUM") as ps2,
            tc.tile_pool(name="ps3", bufs=2, space="PSUM") as ps3,
        ):
# Weights/consts go on the scalar HWDGE queue-set so they don't
            # delay the batch-data stream on the sync queue-set.
            w1m_s = cpool.tile([128, NKC, H1], BF16, tag="w1m")
            nc.scalar.dma_start(w1m_s[:], w1m[:])
            w1t_s = cpool.tile([KT, H1], BF16, tag="w1t")
            nc.scalar.dma_start(w1t_s[:], w1t[:])
            xtl_s = cpool.tile([KT, NT, TN], BF16, tag="xtl")
            nc.scalar.dma_start(xtl_s[:], xt_tail[:])
            b1_s = cpool.tile([H1, 1], F32, tag="b1")
            nc.scalar.dma_start(b1_s[:], b1[:])
            w2_s = cpool.tile([H1, HO], BF16, tag="w2")
            nc.scalar.dma_start(w2_s[:], w2[:])
            b2_s = cpool.tile([HO, 1], F32, tag="b2")
            nc.scalar.dma_start(b2_s[:], b2[:])
            w3_s = cpool.tile([HO, HO], BF16, tag="w3")
            nc.scalar.dma_start(w3_s[:], w3[:])
            b3w_s = cpool.tile([HO, TN], F32, tag="b3w")
            nc.scalar.dma_start(b3w_s[:], b3w[:])

            # Warmup: dense dummy matmuls fill the NEFF startup ramp
            # (~12us of instruction loads + first DMAs) so the PE's HAM
            # throttle reaches full clock before the first real matmul.
            wsc = cpool.tile([128, TN], BF16, tag="wsc")
            nc.gpsimd.memset(wsc[:], 0.0)
            wp0 = ps1.tile([H1, TN], F32, tag="p1")
            wp1 = ps1.tile([H1, TN], F32, tag="p1")
            wp = [wp0, wp1]
            for i in range(24):
                nc.tensor.matmul(
                    wp[i % 2][:], wsc[:, 0:H1], wsc[:],
                    start=True, stop=True,
                )

            # Software pipeline: at step t, emit L1(t), L2(t-2), L3(t-4)
            # so each PE instruction's input was produced >= 2 full
            # L1-blocks earlier — the PE never waits on a fresh ACT relu.
            L2_LAG, L3_LAG = 2, 4
            h1s: dict[int, object] = {}
            h2s: dict[int, object] = {}
            for t in range(NT + L3_LAG):
                if t < NT:
                    xm = iopool.tile([128, NKC, TN], BF16, tag="xm")
                    nc.sync.dma_start(xm[:], xt_main[t])

                    p1 = ps1.tile([H1, TN], F32, tag="p1")
                    for k in range(NKC):
                        nc.tensor.matmul(
                            p1[:], w1m_s[:, k, :], xm[:, k, :],
                            start=(k == 0), stop=False,
                        )
                    nc.tensor.matmul(
                        p1[:], w1t_s[:], xtl_s[:, t, :], start=False, stop=True
                    )
                    h1 = apool.tile([H1, TN], BF16, tag="h1")
                    nc.scalar.activation(h1[:], p1[:], relu, bias=b1_s[:, 0:1])
                    h1s[t] = h1

                if L2_LAG <= t < NT + L2_LAG:
                    s = t - L2_LAG
                    p2 = ps2.tile([HO, TN], F32, tag="p2")
                    nc.tensor.matmul(
                        p2[:], w2_s[:], h1s.pop(s)[:], start=True, stop=True
                    )
                    h2 = apool.tile([HO, TN], BF16, tag="h2")
                    nc.scalar.activation(h2[:], p2[:], relu, bias=b2_s[:, 0:1])
                    h2s[s] = h2

                if t >= L3_LAG:
                    s = t - L3_LAG
                    p3 = ps3.tile([HO, TN], F32, tag="p3")
                    nc.tensor.matmul(
                        p3[:], w3_s[:], h2s.pop(s)[:], start=True, stop=True
                    )
                    ot = apool.tile([HO, TN], F32, tag="ot")
                    nc.vector.tensor_add(ot[:], p3[:], b3w_s[:])
                    nc.sync.dma_start(yt[:, s * TN:(s + 1) * TN], ot[:])

    nc.compile()
    return nc


def _fold_conv_into_w1(conv_w: np.ndarray, w1: np.ndarray) -> np.ndarray:
    """W1f[784,100] such that x @ W1f == conv(x).reshape(B,676) @ w1."""
    c = np.zeros((NF, 26 * 26), dtype=np.float64)
    for di in range(3):
        for dj in range(3):
            ii, jj = np.meshgrid(np.arange(26), np.arange(26), indexing="ij")
            src = (ii + di) * 28 + (jj + dj)
            dst = ii * 26 + jj
            c[src.ravel(), dst.ravel()] += np.float64(conv_w[di, dj])
    return (c @ w1.astype(np.float64)).astype(np.float32)


def _prep_in_maps(x, conv_w, w1, b1, w2, b2, w3, b3):
    x = np.asarray(x, dtype=np.float32)
    conv_w = np.asarray(conv_w, dtype=np.float32)
    w1 = np.asarray(w1, dtype=np.float32)
    b1 = np.asarray(b1, dtype=np.float32)
    w2 = np.asarray(w2, dtype=np.float32)
    b2 = np.asarray(b2, dtype=np.float32)
    w3 = np.asarray(w3, dtype=np.float32)
    b3 = np.asarray(b3, dtype=np.float32)

    w1f = _fold_conv_into_w1(conv_w, w1)  # [784, 100]
    # main chunks: feature f = k*128 + p -> [128, NKC, H1]
    w1m = np.ascontiguousarray(
        w1f[: 128 * NKC].reshape(NKC, 128, H1).transpose(1, 0, 2)
    ).astype(NP_BF16)
    w1t = np.ascontiguousarray(w1f[128 * NKC:]).astype(NP_BF16)  # [16, 100]
    b1c = np.ascontiguousarray(b1.reshape(H1, 1))
    b2c = np.ascontiguousarray(b2.reshape(HO, 1))
    b3w = np.ascontiguousarray(np.broadcast_to(b3.reshape(HO, 1), (HO, TN)))

    shared = {
        "w1m": w1m, "w1t": w1t, "b1": b1c,
        "w2": np.ascontiguousarray(w2).astype(NP_BF16), "b2": b2c,
        "w3": np.ascontiguousarray(w3).astype(NP_BF16), "b3w": b3w,
    }

    xb = x.astype(NP_BF16)  # cast once, full batch
    in_maps = []
    for core in range(N_CORES):
        xc = xb[core * BC:(core + 1) * BC]  # [8192, 784] bf16
        # [NT, TN, NF] -> feature-major per supertile
        xct = xc.reshape(NT, TN, NF).transpose(0, 2, 1)  # [NT, 784, TN]
        xt_main = np.ascontiguousarray(
            xct[:, : 128 * NKC, :].reshape(NT, NKC, 128, TN).transpose(0, 2, 1, 3)
        )  # [NT, 128, NKC, TN]
        # tails of all supertiles together: [KT, NT, TN]
        xt_tail = np.ascontiguousarray(xct[:, 128 * NKC:, :].transpose(1, 0, 2))
        in_maps.append({"xt_main": xt_main, "xt_tail": xt_tail, **shared})
    return in_maps


_NC = None


def _get_nc():
    global _NC
    if _NC is None:
        _NC = _build_nc()
    return _NC


def kernel(x, conv_w, w1, b1, w2, b2, w3, b3):
    in_maps = _prep_in_maps(x, conv_w, w1, b1, w2, b2, w3, b3)
    nc = _get_nc()
    res = run_bass_kernel_spmd(nc, in_maps, core_ids=list(range(N_CORES)))
    out = np.empty((B, HO), dtype=np.float32)
    for i in range(N_CORES):
        out[i * BC:(i + 1) * BC] = res.results[i]["yt"].T
    return out


if __name__ == "__main__":
    rng = np.random.default_rng(0)
    inputs = {
        "x": rng.standard_normal((B, NF), dtype=np.float32),
        "conv_w": np.ones((3, 3), dtype=np.float32),
        "w1": (rng.standard_normal((676, H1)) * 0.04).astype(np.float32),
        "b1": np.zeros(H1, dtype=np.float32),
        "w2": (rng.standard_normal((H1, HO)) * 0.1).astype(np.float32),
        "b2": np.zeros(HO, dtype=np.float32),
        "w3": (rng.standard_normal((HO, HO)) * 0.3).astype(np.float32),
        "b3": np.zeros(HO, dtype=np.float32),
    }
    out = kernel(**inputs)
    print(out.shape, out.dtype)
